# revision 15
# baseline (speedup 1.0000x reference)
"""Transformer block (QKV + causal MHA + proj + GELU-FF, residual) on 8 NeuronCores.

Sharding: DP over batch (2 groups of 4 cores) x TP over heads / FF-inner within
each group; token-chunked bf16 ReduceScatter combines proj+ff partials.

Design notes (vs the ~1.4ms f32r baseline; measured ~0.95-1.1ms):
- Everything bf16 except PSUM accumulation (f32) and the softmax recip
  (f32r). Halves DMA, enables FWL fast weight loads. Measured rel err
  3.5e-3 (budget 2e-2).
- All weights host-retiled so every DMA reads >=4KB contiguous lines.
- x resident in SBUF once (shared by QKV and FF1); wqk (8 tiles) resident;
  w1 streamed once (f-outer over all token chunks). Loads are split across
  the two HWDGE rings (sync=weights, scalar=stores+prefetch) so a
  compute-dependent store can never head-of-line-block a weight load.
- P2 attention software-pipelined (score lookahead 3, finalize deferred 4)
  so PE never waits on the ACT-exp chain. Causal diagonal tiles are
  column-narrowed to their valid range; the boundary mask is added on PE
  via an identity matmul (no DVE in the exp critical path). Softmax
  normalization: PE row-sum (ones matmul), DVE reciprocal, GpSimd
  partition_broadcast, DVE multiply.
- proj+ff2 partials accumulate in one PSUM group; bf16 ReduceScatter per
  (token chunk, 8-row-block piece) -- 2 pieces per chunk so the collective
  starts halfway through each chunk's compute; out-DMA deferred one chunk
  so its RS-wait never blocks a DMA ring.
"""
import numpy as np
import ml_dtypes

import concourse.bass as bass
import concourse.mybir as mybir
import concourse.tile as tile
from concourse import bacc
from concourse import bass_utils

B, T, C = 2, 2048, 2048
H, HD = 16, 128
F = 8192
NCORES = 8
TPG = 4                  # cores per batch group
HPC = H // TPG           # heads per core
QC = 4                   # token chunks per batch
TCH = T // QC            # 512
KT = C // 128            # 16
FPC = F // TPG           # 2048 ff rows per core
FT = FPC // 128          # 16
COT = C // 128           # 16
SM_SCALE = 1.0 / float(np.sqrt(HD))
NEG = -60000.0

f32r = mybir.dt.float32r
f32 = mybir.dt.float32
bf16 = mybir.dt.bfloat16

_CACHED_NC = None


def build_nc(rep=1, rs_mode="split"):
    nc = bacc.Bacc("TRN2", target_bir_lowering=False, debug=False,
                   num_devices=NCORES)
    xb_t = nc.dram_tensor("xb", [128, QC * KT * TCH], bf16,
                          kind="ExternalInput").ap()
    wqk_t = nc.dram_tensor("wqk", [2 * HPC * 128, KT * 128], bf16,
                           kind="ExternalInput").ap()
    wv_t = nc.dram_tensor("wv", [128, KT * 512], bf16,
                          kind="ExternalInput").ap()
    wp_t = nc.dram_tensor("wp", [128, TPG * C], bf16,
                          kind="ExternalInput").ap()
    w1_t = nc.dram_tensor("w1", [FT * 128, KT * 128], bf16,
                          kind="ExternalInput").ap()
    b1_t = nc.dram_tensor("b1", [128, FT], f32, kind="ExternalInput").ap()
    w2_t = nc.dram_tensor("w2", [COT * 128, FT * 128], bf16,
                          kind="ExternalInput").ap()
    out_t = nc.dram_tensor("outp", [C // TPG, T], bf16,
                           kind="ExternalOutput").ap()

    xb_v = xb_t.rearrange("p (c k t) -> p c k t", c=QC, k=KT)
    wqk_v = wqk_t.rearrange("(f p) (k j) -> f p k j", p=128, k=KT)
    wv_v = wv_t.rearrange("p (k j) -> p k j", k=KT)
    wp_v = wp_t.rearrange("p (k j) -> p k j", k=TPG)
    w1_v = w1_t.rearrange("(f p) (k j) -> f p k j", p=128, k=KT)
    w2_v = w2_t.rearrange("(o p) (f j) -> o p f j", p=128, f=FT)

    with tile.TileContext(nc) as tc:
        with tc.tile_pool(name="cstp", bufs=1) as cst, \
             tc.tile_pool(name="dram", bufs=1, space="DRAM") as dram:

            ones_col = cst.tile([128, 1], bf16, name="ones_col", tag="oc")
            nc.gpsimd.memset(ones_col[:], 1.0)
            b1_sb = cst.tile([128, FT], f32, name="b1_sb", tag="b1")
            nc.sync.dma_start(b1_sb[:], b1_t)

            for _rep in range(rep):
              with tc.tile_pool(name="repp", bufs=1) as repp:
                xb_sb = repp.tile([128, QC, KT, TCH], bf16, name="xb_sb",
                                  tag="xb")
                nc.sync.dma_start(xb_sb[:, 0, :, :], xb_v[:, 0, :, :])
                attnT = repp.tile([128, HPC, T], bf16, name="attnT",
                                  tag="attnT")

                # ---------- P1 + P2: qkv and causal attention ----------
                with tc.tile_pool(name="attp", bufs=1) as attp:
                    # boundary masks: one 128-wide block per diagonal
                    # offset d, applied on PE via an identity matmul.
                    masks = attp.tile([128, QC, 128], bf16, name="masks",
                                      tag="mask")
                    nc.gpsimd.memset(masks[:], 0.0)
                    for d in range(QC):
                        nc.gpsimd.affine_select(
                            out=masks[:, d, :], in_=masks[:, d, :],
                            compare_op=mybir.AluOpType.is_ge,
                            fill=NEG, base=0,
                            pattern=[[1, 128]], channel_multiplier=-1,
                        )
                    ident = attp.tile([128, 128], bf16, name="ident",
                                      tag="ident")
                    nc.gpsimd.memset(ident[:], 1.0)
                    nc.gpsimd.affine_select(
                        out=ident[:], in_=ident[:],
                        compare_op=mybir.AluOpType.is_equal,
                        fill=0.0, base=0,
                        pattern=[[1, 128]], channel_multiplier=-1,
                    )
                    qk_sb = attp.tile([128, 2 * HPC, QC, TCH], bf16,
                                      name="qk_sb", tag="qk")
                    v_sb = attp.tile([128, KT, HPC * HD], bf16,
                                     name="v_sb", tag="v")
                    wv_sb = attp.tile([128, KT, 512], bf16, name="wv_sb",
                                      tag="wv")

                    # P1a: q/k feature-major. Pass A covers chunk 0 only
                    # (PE starts after ~2.6MB of DMA); all 8 wqk tiles stay
                    # resident so pass B (chunks 1-3) re-streams nothing.
                    # The xb chunk 1-3 / wv prefetch rides the scalar HWDGE
                    # ring, gated behind pass A's first group by a dummy ACT
                    # copy so it can't steal bandwidth from the critical path.
                    gate = attp.tile([1, 1], f32, name="gate", tag="gate")
                    wqk_tiles = [
                        attp.tile([128, KT, 128], bf16, name=f"wqkt{ft}",
                                  tag=f"wqkt{ft}")
                        for ft in range(2 * HPC)]
                    with tc.tile_pool(name="ps1", bufs=1,
                                      space="PSUM") as ps1:
                      for cs, ce in ((0, 1), (1, QC)):
                        for ft in range(2 * HPC):
                            wqkt = wqk_tiles[ft]
                            if cs == 0:
                                nc.sync.dma_start(wqkt[:], wqk_v[ft])
                            for c in range(cs, ce):
                                pqk = ps1.tile([128, TCH], f32, name="pmm",
                                               tag="pmm", bufs=2)
                                for k in range(KT):
                                    nc.tensor.matmul(
                                        pqk[:], wqkt[:, k, :],
                                        xb_sb[:, c, k, :],
                                        start=(k == 0), stop=(k == KT - 1))
                                nc.vector.tensor_copy(qk_sb[:, ft, c, :],
                                                      pqk[:])
                            if cs == 0 and ft == 0:
                                nc.scalar.activation(
                                    gate[:], qk_sb[0:1, 0, 0, 0:1],
                                    mybir.ActivationFunctionType.Copy)
                                for lc in range(1, QC):
                                    nc.scalar.dma_start(xb_sb[:, lc, :, :],
                                                        xb_v[:, lc, :, :])
                                nc.scalar.dma_start(wv_sb[:], wv_v)

                      # P1b: v token-major (x tiles stationary, wv moving)
                      for c in range(QC):
                        for m in range(TCH // 128):
                            pv = ps1.tile([128, TCH], f32, name="pmm",
                                          tag="pmm", bufs=2)
                            for k in range(KT):
                                nc.tensor.matmul(
                                    pv[:],
                                    xb_sb[:, c, k, m * 128:(m + 1) * 128],
                                    wv_sb[:, k, :],
                                    start=(k == 0), stop=(k == KT - 1))
                            nc.vector.tensor_copy(
                                v_sb[:, c * (TCH // 128) + m, :], pv[:])

                    # P2: causal attention, software-pipelined.
                    ps2_cm = tc.tile_pool(name="ps2", bufs=1,
                                          space="PSUM")
                    ps12 = ps2_cm.__enter__()
                    tasks = []
                    for h in range(HPC):
                        for c in range(QC):
                            nkt = 4 * (c + 1)
                            for kt in range(nkt):
                                tasks.append((h, c, kt, nkt))
                    NT = len(tasks)
                    LOOK = 3      # score lookahead
                    FDELAY = 4    # finalize delay after group's last accum

                    score_bufs = {}
                    e_bufs = {}
                    group_state = {}
                    fin_due = {}  # emit-index -> (h, c)

                    def emit_score(i):
                        h, c, kt, nkt = tasks[i]
                        pscore = ps12.tile([128, TCH], f32, name="pscore",
                                           tag="pscore", bufs=4)
                        kT = qk_sb[:, HPC + h, kt // 4,
                                   (kt % 4) * 128:(kt % 4 + 1) * 128]
                        d = kt - 4 * c
                        if d < 0:
                            nc.tensor.matmul(pscore[:], kT,
                                             qk_sb[:, h, c, :],
                                             start=True, stop=True)
                        else:
                            # diagonal tile: queries < d*128 are fully masked
                            # -- compute only cols [d*128:], and add the
                            # boundary mask on PE (identity matmul).
                            lo = d * 128
                            nc.tensor.matmul(pscore[:, lo:], kT,
                                             qk_sb[:, h, c, lo:],
                                             start=True, stop=False)
                            nc.tensor.matmul(pscore[:, lo:lo + 128], ident[:],
                                             masks[:, d, :],
                                             start=False, stop=True)
                        score_bufs[i] = pscore

                    def emit_exp(i):
                        h, c, kt, nkt = tasks[i]
                        pscore = score_bufs.pop(i)
                        e = attp.tile([128, TCH], bf16, name="e_sb", tag="e",
                                      bufs=5)
                        lo = max(kt - 4 * c, 0) * 128
                        nc.scalar.activation(
                            e[:, lo:], pscore[:, lo:],
                            mybir.ActivationFunctionType.Exp,
                            scale=SM_SCALE)
                        e_bufs[i] = e

                    def emit_accum(i):
                        h, c, kt, nkt = tasks[i]
                        if kt == 0:
                            po = ps12.tile([128, TCH], f32, name="po",
                                           tag="po", bufs=2)
                            psums = ps12.tile([1, TCH], f32, name="psums",
                                              tag="psums", bufs=2)
                            group_state[(h, c)] = (po, psums)
                        po, psums = group_state[(h, c)]
                        e = e_bufs.pop(i)
                        lo = max(kt - 4 * c, 0) * 128
                        nc.tensor.matmul(psums[:, lo:], ones_col[:],
                                         e[:, lo:],
                                         start=(kt == 0), stop=(kt == nkt - 1))
                        nc.tensor.matmul(po[:, lo:],
                                         v_sb[:, kt, h * HD:(h + 1) * HD],
                                         e[:, lo:],
                                         start=(kt == 0), stop=(kt == nkt - 1))

                    def emit_finalize(h, c):
                        po, psums = group_state.pop((h, c))
                        recip = attp.tile([1, TCH], f32r, name="recip",
                                          tag="recip", bufs=2)
                        with nc.allow_low_precision(reason="softmax recip"):
                            nc.vector.reciprocal(recip[:], psums[:])
                        bc_sb = attp.tile([128, TCH], f32r, name="bc_sb",
                                          tag="bc", bufs=2)
                        nc.gpsimd.partition_broadcast(bc_sb[:], recip[:])
                        nc.vector.tensor_mul(
                            attnT[:, h, c * TCH:(c + 1) * TCH],
                            po[:], bc_sb[:])

                    for i in range(NT):
                        emit_score(i)
                        emit_exp(i)
                        j = i - LOOK
                        if j >= 0:
                            emit_accum(j)
                            h, c, kt, nkt = tasks[j]
                            if kt == nkt - 1:
                                fin_due[j + FDELAY] = (h, c)
                        if i in fin_due:
                            emit_finalize(*fin_due.pop(i))
                    for j in range(max(NT - LOOK, 0), NT):
                        emit_accum(j)
                        h, c, kt, nkt = tasks[j]
                        if kt == nkt - 1:
                            fin_due[j + FDELAY] = (h, c)
                    for j in sorted(fin_due):
                        emit_finalize(*fin_due.pop(j))
                    ps2_cm.__exit__(None, None, None)

                # ---------- P3: FF1 (f-outer), then proj+FF2 + RS ----------
                with tc.tile_pool(name="p3w", bufs=1) as p3w, \
                     tc.tile_pool(name="ps3", bufs=1, space="PSUM") as ps3:
                    h_sb = p3w.tile([128, FT, QC, TCH], bf16, name="h_sb",
                                    tag="h")
                    wp_sb = p3w.tile([128, TPG, C], bf16, name="wp_sb",
                                     tag="wp")
                    nc.scalar.dma_start(wp_sb[:], wp_v)

                    for f in range(FT):
                        w1t = repp.tile([128, KT, 128], bf16, name="w1t",
                                        tag="w1t", bufs=3)
                        nc.sync.dma_start(w1t[:], w1_v[f])
                        phs = [ps3.tile([128, TCH], f32, name="ph",
                                        tag=f"ph{c}", bufs=1)
                               for c in range(QC)]
                        for k in range(KT):
                            for c in range(QC):
                                nc.tensor.matmul(
                                    phs[c][:], w1t[:, k, :],
                                    xb_sb[:, c, k, :],
                                    start=(k == 0), stop=(k == KT - 1))
                        for c in range(QC):
                            nc.scalar.activation(
                                h_sb[:, f, c, :], phs[c][:],
                                mybir.ActivationFunctionType.Gelu,
                                bias=b1_sb[:, f:f + 1], scale=1.0)

                    rs_pending = []
                    for c in range(QC):
                        rs_out = dram.tile([C // TPG, TCH], bf16,
                                           name="rs_out", tag="rso", bufs=2)
                        for pi, cos in enumerate(
                                ([0, 1, 4, 5, 8, 9, 12, 13],
                                 [2, 3, 6, 7, 10, 11, 14, 15])):
                            rs_in = dram.tile([TPG * 256, TCH], bf16,
                                              name="rs_in", tag="rsi",
                                              bufs=4)
                            for co in cos:
                                w2t = p3w.tile([128, FT, 128], bf16,
                                               name="w2t", tag="w2t", bufs=3)
                                nc.sync.dma_start(w2t[:], w2_v[co])
                                pout = ps3.tile([128, TCH], f32, name="pout",
                                                tag="pout", bufs=2)
                                for k4 in range(TPG):
                                    nc.tensor.matmul(
                                        pout[:],
                                        wp_sb[:, k4, co * 128:(co + 1) * 128],
                                        attnT[:, k4, c * TCH:(c + 1) * TCH],
                                        start=(k4 == 0), stop=False)
                                for f in range(FT):
                                    nc.tensor.matmul(
                                        pout[:], w2t[:, f, :],
                                        h_sb[:, f, c, :],
                                        start=False, stop=(f == FT - 1))
                                o_sb = p3w.tile([128, TCH], bf16, name="o_sb",
                                                tag="o", bufs=3)
                                nc.vector.tensor_copy(o_sb[:], pout[:])
                                ro = (co // 4) * 256 + (co % 2) * 128
                                nc.scalar.dma_start(
                                    rs_in[ro:ro + 128, :], o_sb[:])
                            if rs_mode == "split":
                                nc.gpsimd.collective_compute(
                                    "ReduceScatter", mybir.AluOpType.add,
                                    replica_groups=[[0, 1, 2, 3],
                                                    [4, 5, 6, 7]],
                                    ins=[rs_in.opt()],
                                    outs=[rs_out[pi * 256:
                                                 (pi + 1) * 256, :].opt()])
                            else:
                                nc.sync.dma_start(
                                    rs_out[pi * 256:(pi + 1) * 256, :],
                                    rs_in[0:256, :])
                        rs_pending.append((c, rs_out))
                        if c > 0:
                            pc, prs = rs_pending.pop(0)
                            nc.scalar.dma_start(
                                out_t[:, pc * TCH:(pc + 1) * TCH], prs[:])
                    for pc, prs in rs_pending:
                        nc.scalar.dma_start(
                            out_t[:, pc * TCH:(pc + 1) * TCH], prs[:])

    nc.compile()
    return nc


def make_in_maps(x, w_qkv, w_proj, w_ff1, b_ff1, w_ff2):
    in_maps = []
    asc = np.ascontiguousarray
    bf = ml_dtypes.bfloat16
    for r in range(NCORES):
        b, hg = r // TPG, r % TPG
        xT = asc(x[b].T).astype(bf)                      # [C, T]
        xb_h = asc(xT.reshape(KT, 128, QC, TCH).transpose(1, 2, 0, 3)
                   ).reshape(128, QC * KT * TCH)
        q_cols = w_qkv[:, hg * 512:(hg + 1) * 512]
        k_cols = w_qkv[:, C + hg * 512:C + (hg + 1) * 512]
        v_cols = w_qkv[:, 2 * C + hg * 512:2 * C + (hg + 1) * 512]
        qk = np.concatenate([q_cols, k_cols], axis=1).astype(bf)  # [C, 1024]
        wqk_h = asc(qk.reshape(KT, 128, 2 * HPC, 128).transpose(2, 1, 0, 3)
                    ).reshape(2 * HPC * 128, KT * 128)
        wv_h = asc(v_cols.astype(bf).reshape(KT, 128, 512).transpose(1, 0, 2)
                   ).reshape(128, KT * 512)
        wp_rows = w_proj[hg * 512:(hg + 1) * 512, :].astype(bf)   # [512, C]
        wp_h = asc(wp_rows.reshape(TPG, 128, C).transpose(1, 0, 2)
                   ).reshape(128, TPG * C)
        w1s = w_ff1[:, hg * FPC:(hg + 1) * FPC].astype(bf)        # [C, 2048]
        w1_h = asc(w1s.reshape(KT, 128, FT, 128).transpose(2, 1, 0, 3)
                   ).reshape(FT * 128, KT * 128)
        w2s = w_ff2[hg * FPC:(hg + 1) * FPC, :].astype(bf)        # [2048, C]
        w2_h = asc(w2s.reshape(FT, 128, COT, 128).transpose(2, 1, 0, 3)
                   ).reshape(COT * 128, FT * 128)
        in_maps.append({
            "xb": xb_h,
            "wqk": wqk_h,
            "wv": wv_h,
            "wp": wp_h,
            "w1": w1_h,
            "b1": asc(b_ff1[hg * FPC:(hg + 1) * FPC].reshape(FT, 128).T),
            "w2": w2_h,
        })
    return in_maps


def assemble(results, x, b_ff2):
    out = np.empty((B, T, C), np.float32)
    for r in range(NCORES):
        b, idx = r // TPG, r % TPG
        out[b, :, idx * 512:(idx + 1) * 512] = \
            results[r]["outp"].T.astype(np.float32)
    out += x + b_ff2
    return out


def kernel(x, w_qkv, w_proj, w_ff1, b_ff1, w_ff2, b_ff2):
    global _CACHED_NC
    x = np.asarray(x, np.float32)
    if _CACHED_NC is None:
        _CACHED_NC = build_nc()
    in_maps = make_in_maps(x, np.asarray(w_qkv, np.float32),
                           np.asarray(w_proj, np.float32),
                           np.asarray(w_ff1, np.float32),
                           np.asarray(b_ff1, np.float32),
                           np.asarray(w_ff2, np.float32))
    res = bass_utils.run_bass_kernel_spmd(_CACHED_NC, in_maps,
                                          core_ids=list(range(NCORES)))
    return assemble(res.results, x, np.asarray(b_ff2, np.float32))
